# revision 18
# baseline (speedup 1.0000x reference)
"""Single-head causal attention on 8 Trainium2 NeuronCores (Bass/Tile).

x:[4,4096,1024] f32, Wq/Wk/Wv:[1024,64] f32 -> out:[4,4096,64] f32.

Strategy (hardcoded, self-contained):
- Sequence-parallel with balanced chunk pairing: T=4096 split into 16
  chunks of 256; core r owns query chunks (r, 15-r) of every batch ->
  equal causal score work per core.
- Each core computes Q^T/K^T (h-major) and V (token-major) for its own
  2048 tokens from its xT shard (bf16); K^T/V are shared via four
  per-batch AllGathers that overlap projections and attention. A dummy
  warm-up AllGather at kernel start absorbs the one-time CC barrier.
- Attention in S^T orientation: S^T[s,q] = K^T-stationary x Q^T-moving;
  exp on ScalarE (no max subtraction -- scores*C^-0.5 are O(1) for this
  input distribution); AV with [V|1] stationary so row 64 of the
  accumulator is sumexp; normalization + transpose on host.
- One SPMD program for all 8 cores: the per-core loop structure is
  encoded in a 51-entry uint32 schedule table loaded once into engine
  registers, driving dynamic (register-offset) APs; the causal diagonal
  lands at fixed iterations (0 and 16) so masking is static. K/V
  atlases are laid out in (slot, source-core) order so each gather is
  two strided DMAs.
"""

import sys

import numpy as np

sys.path.insert(0, "/opt/trn_rl_repo")
import ml_dtypes  # noqa: E402
from concourse import bass, bacc, tile, mybir  # noqa: E402
from concourse.bass_utils import run_bass_kernel_spmd  # noqa: E402

BF16 = mybir.dt.bfloat16
F32 = mybir.dt.float32
U32 = mybir.dt.uint32
PE = mybir.EngineType.PE
DVE = mybir.EngineType.DVE
Exp = mybir.ActivationFunctionType.Exp
Copy = mybir.ActivationFunctionType.Copy

B, T, C, H = 4, 4096, 1024, 64
R = 8                     # cores
CH = 256                  # query/key chunk
NCH = T // CH             # 16 chunks
NTOK = B * 2 * CH         # 2048 tokens owned per core
BTOK = 2 * CH             # 512 tokens per (core, batch)
KELEM = H * BTOK          # 32768 elements in per-batch K^T shard [64, 512]
SHARD = 2 * KELEM         # 65536: K^T + V per batch
NIT = NCH + 1             # 17 iterations per batch (uniform across cores)
SCALE = float(C) ** -0.5
VW = H + 1                # V tile width incl ones column
VB = 2 * R * VW           # vatl elements per slot block (16 tiles x 65)

_built = None


def _build():
    global _built
    if _built is not None:
        return _built

    nc = bacc.Bacc("TRN2", target_bir_lowering=False, debug=False, num_devices=R)

    xT_d = nc.dram_tensor("xT", [C, NTOK], BF16, kind="ExternalInput")
    wqk_d = nc.dram_tensor("wqk", [C, 128], BF16, kind="ExternalInput")
    wv_d = nc.dram_tensor("wv", [C, H], BF16, kind="ExternalInput")
    mask_d = nc.dram_tensor("maskd", [128, 2 * CH], BF16, kind="ExternalInput")
    tab_d = nc.dram_tensor("regtab", [1, 3 * NIT], U32, kind="ExternalInput")
    out_d = nc.dram_tensor("outp", [B, VW, 2 * CH], F32, kind="ExternalOutput")

    ag_in = [
        nc.dram_tensor(f"ag_in{b}", [1, SHARD], BF16, kind="Internal")
        for b in range(B)
    ]
    ag_out = [
        nc.dram_tensor(
            f"ag_out{b}", [R, SHARD], BF16, kind="Internal", addr_space="Shared"
        )
        for b in range(B)
    ]
    agw_in = nc.dram_tensor("agw_in", [1, 16], BF16, kind="Internal")
    agw_out = nc.dram_tensor(
        "agw_out", [R, 16], BF16, kind="Internal", addr_space="Shared"
    )

    with tile.TileContext(nc) as tc:
        with tc.tile_pool(name="outer", bufs=1) as outer:
            # CC warm-up: tiny AllGather issued first (high priority) so the
            # one-time collectives barrier overlaps the projection phase.
            with tc.high_priority():
                nc.sync.dma_start(agw_in[:], mask_d[0:1, 0:16])
                nc.gpsimd.collective_compute(
                    "AllGather",
                    mybir.AluOpType.bypass,
                    replica_groups=[list(range(R))],
                    ins=[agw_in[:]],
                    outs=[agw_out[:]],
                )

            qT = outer.tile([H, NTOK], BF16)
            kshard = outer.tile([H, NTOK], BF16)
            vshard = outer.tile([128, (NTOK // 128) * H], BF16)
            maskt = outer.tile([128, 2 * CH], BF16)
            tabt = outer.tile([1, 3 * NIT], U32)
            zero65 = outer.tile([128, VW], BF16)

            nc.sync.dma_start(maskt[:], mask_d[:])
            nc.sync.dma_start(tabt[:], tab_d[:])

            # persistent V atlases (2 rotating slots, ones column written once)
            vatlA = outer.tile([128, 2 * VB], BF16)
            vatlB = outer.tile([128, 2 * VB], BF16)
            with tc.high_priority():
                nc.gpsimd.memset(zero65[:], 0.0)
                nc.gpsimd.memset(vatlA[:], 1.0)
                nc.gpsimd.memset(vatlB[:], 1.0)

            # schedule registers, loaded once: PE outsel[i], DVE koff[i]/voff[i]
            rpe = [nc.alloc_register(PE, f"rpe{i}") for i in range(NIT)]
            nc.reg_load(rpe, tabt[0:1, 0:NIT])
            sv_o = [
                nc.snap(r, donate=True, min_val=0, max_val=CH) for r in rpe
            ]
            rdk = [nc.alloc_register(DVE, f"rdk{i}") for i in range(NIT)]
            nc.reg_load(rdk, tabt[0:1, NIT : 2 * NIT])
            sv_k = [
                nc.snap(r, donate=True, min_val=0, max_val=T - CH) for r in rdk
            ]
            rdv = [nc.alloc_register(DVE, f"rdv{i}") for i in range(NIT)]
            nc.reg_load(rdv, tabt[0:1, 2 * NIT : 3 * NIT])
            sv_v = [
                nc.snap(r, donate=True, min_val=0, max_val=2 * VB - 2 * VW)
                for r in rdv
            ]

            # ---------------- projections + per-batch allgather ----------
            with (
                tc.tile_pool(name="proj", bufs=1) as pj,
                tc.tile_pool(name="pjps", bufs=2, space="PSUM") as pjps,
            ):
                xts_all = pj.tile([128, 8 * NTOK], BF16)
                for k in range(8):
                    nc.sync.dma_start(
                        xts_all[:, k * NTOK : (k + 1) * NTOK],
                        xT_d[k * 128 : (k + 1) * 128, :],
                    )
                wqk_all = pj.tile([128, 8 * 128], BF16)
                nc.sync.dma_start(
                    wqk_all[:].rearrange("p (k c) -> p k c", k=8),
                    wqk_d[:].rearrange("(k p) c -> p k c", k=8),
                )
                wv_all = pj.tile([128, 8 * H], BF16)
                nc.sync.dma_start(
                    wv_all[:].rearrange("p (k c) -> p k c", k=8),
                    wv_d[:].rearrange("(k p) c -> p k c", k=8),
                )
                xts = [xts_all[:, k * NTOK : (k + 1) * NTOK] for k in range(8)]
                wqks = [wqk_all[:, k * 128 : (k + 1) * 128] for k in range(8)]
                wvs = [wv_all[:, k * H : (k + 1) * H] for k in range(8)]

                for b in range(B):
                    # Q^T rows 0:64 + K^T rows 64:128 for this batch's tokens
                    ps = pjps.tile([128, BTOK], F32, tag="psqk")
                    for k in range(8):
                        nc.tensor.matmul(
                            ps[:],
                            wqks[k],
                            xts[k][:, b * BTOK : (b + 1) * BTOK],
                            start=(k == 0),
                            stop=(k == 7),
                        )
                    nc.scalar.activation(
                        qT[:, b * BTOK : (b + 1) * BTOK], ps[0:H, :], Copy
                    )
                    nc.scalar.activation(
                        kshard[:, b * BTOK : (b + 1) * BTOK], ps[H:128, :], Copy
                    )
                    # V token-major for this batch (4 tiles of 128 tokens)
                    for q in range(4):
                        tt = 4 * b + q
                        psv = pjps.tile([128, H], F32, tag="psv")
                        for k in range(8):
                            nc.tensor.matmul(
                                psv[:],
                                xts[k][:, tt * 128 : (tt + 1) * 128],
                                wvs[k],
                                start=(k == 0),
                                stop=(k == 7),
                            )
                        nc.scalar.activation(
                            vshard[:, tt * H : (tt + 1) * H], psv[:], Copy
                        )
                    # shard -> DRAM -> per-batch AllGather
                    nc.sync.dma_start(
                        ag_in[b][0:1, 0:KELEM].rearrange("1 (h t) -> h t", h=H),
                        kshard[:, b * BTOK : (b + 1) * BTOK],
                    )
                    nc.sync.dma_start(
                        ag_in[b][0:1, KELEM:SHARD].rearrange(
                            "1 (tt p h) -> p tt h", tt=4, p=128
                        ),
                        vshard[:, 4 * b * H : (4 * b + 4) * H].rearrange(
                            "p (tt h) -> p tt h", tt=4
                        ),
                    )
                    nc.gpsimd.collective_compute(
                        "AllGather",
                        mybir.AluOpType.bypass,
                        replica_groups=[list(range(R))],
                        ins=[ag_in[b][:]],
                        outs=[ag_out[b][:]],
                    )

            # ---------------- attention ----------------
            # atlases in (slot, source-core) order:
            #   katl col  sl*2048 + rc*256 + t
            #   vatl tile (sl*16 + rc*2 + u) of width VW (data cols 0:64, ones col 64)
            with (
                tc.tile_pool(name="atl", bufs=2) as atl,
                tc.tile_pool(name="ptp", bufs=3) as ptp,
                tc.tile_pool(name="scps", bufs=2, space="PSUM") as scps_p,
                tc.tile_pool(name="acps", bufs=2, space="PSUM") as acps_p,
                tc.tile_pool(name="outb", bufs=2) as outb_p,
            ):
                pending = None  # (accum, b) awaiting psum->sbuf->dram flush

                def flush_pending():
                    nonlocal pending
                    if pending is None:
                        return
                    accum_prev, b_prev = pending
                    outsb = outb_p.tile([VW, 2 * CH], F32, tag="outsb")
                    nc.vector.tensor_copy(outsb[:], accum_prev[:])
                    nc.sync.dma_start(out_d[b_prev], outsb[:])
                    pending = None

                for b in range(B):
                    katl = atl.tile([H, T], BF16, tag="katl")
                    vatl = vatlA if b % 2 == 0 else vatlB
                    for sl in range(2):
                        # K: src [rc, h, 256] -> dst [h, rc, 256]
                        nc.sync.dma_start(
                            katl[:, sl * 8 * CH : (sl + 1) * 8 * CH].rearrange(
                                "h (rc t) -> h rc t", rc=R
                            ),
                            ag_out[b][:, 0:KELEM]
                            .rearrange("rc (h t) -> h rc t", h=H)[
                                :, :, sl * CH : (sl + 1) * CH
                            ],
                        )
                        # V: src [rc, p, h] -> dst [p, rc, h], one DMA per u
                        for u in range(2):
                            nc.sync.dma_start(
                                vatl[:, sl * VB : (sl + 1) * VB]
                                .rearrange("p (rc u w) -> p rc u w", rc=R, u=2)[
                                    :, :, u, 0:H
                                ],
                                ag_out[b][:, KELEM:SHARD]
                                .rearrange("rc (u p h) -> p rc u h", u=4, p=128)[
                                    :, :, 2 * sl + u, :
                                ],
                            )

                    accum = acps_p.tile([VW, 2 * CH], F32, tag="accum")
                    # clear has_written over the whole bank with a zero matmul
                    nc.tensor.matmul(
                        accum[:], zero65[:, 0:VW], maskt[:], start=True, stop=False
                    )

                    for pair in range(9):
                        iters = [2 * pair] if pair == 8 else [2 * pair, 2 * pair + 1]
                        scps = scps_p.tile([128, 1024], F32, tag="scores")
                        ptile = ptp.tile([128, 1024], BF16, tag="ptile")
                        stg = []
                        for i in iters:
                            col0 = (i % 2) * 512
                            kst = ptp.tile([H, CH], BF16, tag="kst")
                            nc.vector.tensor_copy(
                                kst[:], katl[:, bass.ds(sv_k[i], CH)]
                            )
                            vst = ptp.tile([128, 2 * VW], BF16, tag="vst")
                            nc.vector.tensor_copy(
                                vst[:], vatl[:, bass.ds(sv_v[i], 2 * VW)]
                            )
                            stg.append((kst, vst))
                            qslice = qT[:, b * BTOK : (b + 1) * BTOK]
                            nc.tensor.matmul(
                                scps[:, col0 : col0 + CH],
                                kst[:, 0:128],
                                qslice[:, bass.ds(sv_o[i], CH)],
                                start=True,
                                stop=True,
                            )
                            nc.tensor.matmul(
                                scps[:, col0 + CH : col0 + 2 * CH],
                                kst[:, 128:256],
                                qslice[:, bass.ds(sv_o[i], CH)],
                                start=True,
                                stop=True,
                            )
                        if pair == 8:
                            nc.scalar.activation(
                                ptile[:, 0:512], scps[:, 0:512], Exp, scale=SCALE
                            )
                        else:
                            nc.scalar.activation(ptile[:], scps[:], Exp, scale=SCALE)
                        if pair in (0, 8):
                            # causal diagonal of slot-0 chunk (pair 0) or
                            # slot-1 chunk (pair 8, i==16)
                            nc.vector.tensor_mul(
                                ptile[:, 0:512], ptile[:, 0:512], maskt[:]
                            )
                        for j, i in enumerate(iters):
                            col0 = (i % 2) * 512
                            kst, vst = stg[j]
                            nc.tensor.matmul(
                                accum[0:VW, bass.ds(sv_o[i], CH)],
                                vst[:, 0:VW],
                                ptile[:, col0 : col0 + CH],
                                start=False,
                                stop=False,
                            )
                            nc.tensor.matmul(
                                accum[0:VW, bass.ds(sv_o[i], CH)],
                                vst[:, VW : 2 * VW],
                                ptile[:, col0 + CH : col0 + 2 * CH],
                                start=False,
                                stop=(i == 16),
                            )
                        if pair == 1:
                            flush_pending()
                    pending = (accum, b)
                flush_pending()

    nc.compile()
    _built = nc
    return nc


def _chunk_home(c):
    """chunk c of any batch lives on core rc at slot sl."""
    return (c, 0) if c < R else (15 - c, 1)


def _prep_inputs(x, Wq, Wk, Wv):
    bf = ml_dtypes.bfloat16
    x = np.asarray(x, np.float32)
    wqk = np.concatenate(
        [np.asarray(Wq, np.float32), np.asarray(Wk, np.float32)], axis=1
    ).astype(bf)
    wv = np.asarray(Wv, np.float32).astype(bf)

    s_idx = np.arange(128)[:, None]
    q_idx = np.arange(CH)[None, :]
    maskd = np.concatenate(
        [(q_idx >= s_idx), (q_idx >= s_idx + 128)], axis=1
    ).astype(bf)

    in_maps = []
    for r in range(R):
        c1, c2 = r, 15 - r
        rows = []
        for b in range(B):
            rows.append(x[b, c1 * CH : (c1 + 1) * CH, :])
            rows.append(x[b, c2 * CH : (c2 + 1) * CH, :])
        xs = np.concatenate(rows, axis=0)  # [2048, 1024]
        xT = np.ascontiguousarray(xs.T).astype(bf)  # [1024, 2048]

        outsel, koff, voff = [], [], []
        for i in range(NIT):
            s = 0 if i <= c1 else 1
            sc = c1 - i if i <= c1 else i - c1 - 1
            rc, sl = _chunk_home(sc)
            outsel.append(s * CH)
            koff.append(sl * 8 * CH + rc * CH)
            voff.append(sl * VB + rc * 2 * VW)
        tab = np.asarray(outsel + koff + voff, np.uint32)[None, :]
        in_maps.append(
            {"xT": xT, "wqk": wqk, "wv": wv, "maskd": maskd, "regtab": tab}
        )
    return in_maps


def _assemble(results):
    out = np.empty((B, T, H), np.float32)
    for r in range(R):
        o = results[r]["outp"]  # [B, 65, 512]
        for b in range(B):
            for sl, c in ((0, r), (1, 15 - r)):
                blk = o[b, :, sl * CH : (sl + 1) * CH]
                out[b, c * CH : (c + 1) * CH, :] = (blk[0:H] / blk[H : H + 1]).T
    return out


def run_raw(x, Wq, Wk, Wv, **kwargs):
    nc = _build()
    in_maps = _prep_inputs(x, Wq, Wk, Wv)
    return run_bass_kernel_spmd(nc, in_maps, core_ids=list(range(R)), **kwargs)


def kernel(x, Wq, Wk, Wv):
    res = run_raw(x, Wq, Wk, Wv)
    return _assemble(res.results)
